# revision 42
# baseline (speedup 1.0000x reference)
"""Behler-Parrinello NN potential kernel for 8x Trainium2 NeuronCores.

Strategy (moe_routing; the kernel is ACT(tanh)-bound, so everything is
arranged around keeping the scalar engine 100% fed while the tensor
engine's work is cut below it with fp8 DoubleRow):

  - Host: partition atoms by type, pad each per-core type group to a
    multiple of 128, shard across 8 cores (data-parallel over atoms,
    per-type MLP weights replicated). Feature-major fp16 Gs slices.
  - Residual-form L2 (the accuracy trick that makes fp8 viable): with
    c = E[z tanh z] for z~N(0,1), write tanh(z) = c*z - s(z) where
    s has RMS 0.166 (vs 0.63 for tanh). Then
        z2 = h1 @ W2 + b2 = G @ (c*W1@W2) - s @ W2 + b2
    The big linear term is an exact fp16 K=128 matmul against the
    host-precomputed W12 = 64*c*W1@W2; only the small residual s goes
    through the fp8e4 DoubleRow matmul (K=256/pass, 0.5 cyc/row), so
    both fp8 quantization error sources shrink ~4x (sim: 1.3e-2 rel
    metric vs 4.3e-2 for plain fp8; gate is 2e-2).
  - Per 512-atom block: L1 z1' = G@(c*W1) (fp16) -> ACT tanh (scale
    1/c) -> DVE s = z1' - h1 (writes fp8) -> L2 z2 = G@W12 - s@W2q
    (fp16 + fp8 DoubleRow accumulating in PSUM) -> ACT tanh (scale
    1/64) -> L3 fp16 col-tiled 4-up -> DVE e-copy -> DMA out.
  - PSUM (8 banks exactly): z1 [128,2,512] x2 bufs (4 banks) + z2
    [128,2,512] x1 (2) + e [128,512] x2 (2). ACT runs FD=1024 calls on
    m-group pairs; the z1 double-buffer breaks what would otherwise be
    a serializing ACT1 -> DVE-sub -> L1(next) cycle.
  - 3-stage software pipeline across blocks: L1(i) | L2(i-1) | L3(i-2).
"""

import sys
import time

sys.path.insert(0, "/opt/trn_rl_repo")

import numpy as np
import ml_dtypes

import concourse.bacc as bacc
import concourse.mybir as mybir
from concourse import tile
from concourse.bass_utils import run_bass_kernel_spmd

N_CORES = 8
NUM_GS = 128
HIDDEN = 512
N_MOL = 1024
BLK = 512            # atoms per block (one z tile = 2 PSUM banks)
MCH = HIDDEN // 128  # hidden chunks of 128 (partition tiles)

F32 = mybir.dt.float32
F16 = mybir.dt.float16
F8 = mybir.dt.float8e4
DR = mybir.MatmulPerfMode.DoubleRow

# c = E[z*tanh(z)], z~N(0,1): the L2-optimal linear coefficient of tanh.
C_LIN = 0.6054352892808054
W2SC = 64.0          # power-of-2 scale keeping fp8 W2 out of subnormals

_PROGRAM_CACHE: dict = {}


def _q8(x):
    """TRN FP8_EXP4 (e4m3, max +-240) quantization on host."""
    return np.clip(np.asarray(x, np.float32), -240.0, 240.0).astype(
        ml_dtypes.float8_e4m3)


def _emit(nc, n_a: int, n_b: int, zero_bias: bool):
    """Emit the SPMD Bass IR: n_a A-atoms + n_b B-atoms per core."""
    ntot = n_a + n_b

    gst = nc.dram_tensor("gst", [NUM_GS, ntot], F16, kind="ExternalInput")
    e_out = nc.dram_tensor("e_out", [MCH, ntot], F32, kind="ExternalOutput")
    dram = {}
    for t in ("a", "b"):
        dram[f"w1{t}"] = nc.dram_tensor(f"w1{t}", [NUM_GS, HIDDEN], F16,
                                        kind="ExternalInput")
        dram[f"w12{t}"] = nc.dram_tensor(f"w12{t}", [NUM_GS, HIDDEN], F16,
                                         kind="ExternalInput")
        for k2 in range(2):
            dram[f"w2q{k2}{t}"] = nc.dram_tensor(
                f"w2q{k2}{t}", [128, 2, HIDDEN], F8, kind="ExternalInput")
        dram[f"w3{t}"] = nc.dram_tensor(f"w3{t}", [128, 32 * MCH], F16,
                                        kind="ExternalInput")
        if not zero_bias:
            dram[f"b1{t}"] = nc.dram_tensor(f"b1{t}", [128, MCH], F32,
                                            kind="ExternalInput")
            dram[f"b2{t}"] = nc.dram_tensor(f"b2{t}", [128, MCH], F32,
                                            kind="ExternalInput")

    Tanh = mybir.ActivationFunctionType.Tanh
    inv_c = float(1.0 / C_LIN)
    inv_w2sc = float(1.0 / W2SC)

    with tile.TileContext(nc) as tc:
        with (
            tc.tile_pool(name="wpool", bufs=1) as wpool,
            tc.tile_pool(name="gpool", bufs=5) as gpool,
            tc.tile_pool(name="h1pool", bufs=4) as h1pool,
            tc.tile_pool(name="spool", bufs=4) as spool,
            tc.tile_pool(name="h2pool", bufs=4) as h2pool,
            tc.tile_pool(name="epool", bufs=3) as epool,
            tc.tile_pool(name="z1pool", bufs=2, space="PSUM") as z1pool,
            tc.tile_pool(name="z2pool", bufs=2, space="PSUM") as z2pool,
        ):
            # Warm the PE (HAM clock gate) with matmuls on scratch SBUF
            # while the first DMAs are in flight; result never read.
            scratch = wpool.tile([128, 512], F16, tag="scratch")
            nc.gpsimd.memset(scratch[:, :], 0)
            wps = z2pool.tile([128, 2, BLK], F32, tag="z2")
            for i in range(5):
                nc.tensor.matmul(wps[:, 0, :], scratch[:, 0:128], scratch[:, :],
                                 start=(i == 0), stop=(i == 4))
            # Filler matmuls: scheduled (by the manifest transform) into
            # the pipeline-fill idle window after L1(0), where the PE
            # otherwise sits ~2.4us waiting on the first sub and drops
            # out of its boosted p-state -- every later matmul then runs
            # ~630ns instead of ~380ns until the clock re-ramps.
            for j in range(14):
                nc.tensor.matmul(wps[:, 0, 0:128], scratch[:, 0:128],
                                 scratch[:, 0:128],
                                 start=(j == 0), stop=(j == 13))

            # Weight DMA plan: the DMA engines round-robin everything
            # in flight, so bulk weight traffic steals bandwidth from
            # gs(0), which gates the first real matmul. w1a rides the
            # sync queue as ONE dma ahead of gs0/gs1 (prefetched below);
            # the other A weights follow on gpsimd; the B-type weights
            # (~480KB, first needed ~70us in) are gated behind a dummy
            # read of gs0 so they only start pulling HBM afterwards.
            sb = {}
            for t in ("a", "b"):
                w1 = wpool.tile([128, HIDDEN], F16, tag=f"w1{t}")
                w12 = wpool.tile([128, HIDDEN], F16, tag=f"w12{t}")
                w2q = []
                for k2 in range(2):
                    w2k = wpool.tile([128, 2, HIDDEN], F8, tag=f"w2q{k2}{t}")
                    w2q.append(w2k)
                w3 = wpool.tile([128, 32 * MCH], F16, tag=f"w3{t}")
                if zero_bias:
                    b1 = b2 = None
                else:
                    b1 = wpool.tile([128, MCH], F32, tag=f"b1{t}")
                    b2 = wpool.tile([128, MCH], F32, tag=f"b2{t}")
                sb[t] = (w1, w12, w2q, w3, b1, b2)

            def emit_weight_dmas(t, eng):
                w1, w12, w2q, w3, b1, b2 = sb[t]
                if t == "b":
                    eng.dma_start(w1[:, :], dram[f"w1{t}"][:, :])
                eng.dma_start(w12[:, :], dram[f"w12{t}"][:, :])
                for k2 in range(2):
                    eng.dma_start(w2q[k2][:, :, :],
                                  dram[f"w2q{k2}{t}"][:, :, :])
                eng.dma_start(w3[:, :], dram[f"w3{t}"][:, :])
                if not zero_bias:
                    eng.dma_start(b1[:, :], dram[f"b1{t}"][:, :])
                    eng.dma_start(b2[:, :], dram[f"b2{t}"][:, :])

            nc.sync.dma_start(sb["a"][0][:, :], dram["w1a"][:, :])

            # Block schedule: contiguous A atoms, then B atoms; blocks of
            # BLK with a 128-multiple remainder. Keep the very last block
            # small so the exit drain chain is short.
            blocks = []
            off = 0
            for t, n_at in (("a", n_a), ("b", n_b)):
                rem = n_at
                while rem:
                    w = min(BLK, rem)
                    blocks.append((t, off, w))
                    off += w
                    rem -= w
            if blocks and blocks[-1][2] > 128:
                t, boff, w = blocks[-1]
                blocks[-1] = (t, boff, w - 128)
                blocks.append((t, boff + w - 128, 128))

            gs_of, h1_of, s_of, h2_of = {}, {}, {}, {}

            z2_of = {}

            def emit_gs_dma(bi):
                ex, boff, w = blocks[bi]
                gs = gpool.tile([128, BLK], F16, tag="gs")
                nc.sync.dma_start(gs[:, 0:w], gst[:, boff:boff + w])
                gs_of[bi] = gs

            # gs0/gs1 right behind w1a on the sync queue; A-type L2/L3
            # weights next on gpsimd; B weights gated on gs0's arrival.
            emit_gs_dma(0)
            if len(blocks) > 1:
                emit_gs_dma(1)
            emit_weight_dmas("a", nc.gpsimd)
            gate = wpool.tile([1, 8], F16, tag="gate")
            nc.gpsimd.tensor_copy(gate[:, :], gs_of[0][0:1, 0:8])
            emit_weight_dmas("b", nc.gpsimd)

            def emit_l1_pair(bi, pair):
                ex, boff, w = blocks[bi]
                w1, _, _, _, b1, _ = sb[ex]
                if pair == 0:
                    if bi not in gs_of:
                        emit_gs_dma(bi)
                    gs = gs_of[bi]
                    h1 = h1pool.tile([128, MCH, BLK], F16, tag="h1")
                    s = spool.tile([128, MCH, BLK], F8, tag="s")
                    h1_of[bi] = h1
                    s_of[bi] = s
                gs, h1, s = gs_of[bi], h1_of[bi], s_of[bi]
                mlo = 2 * pair
                z1 = z1pool.tile([128, 2, BLK], F32, tag="z1")
                for g in range(2):
                    m = mlo + g
                    nc.tensor.matmul(z1[:, g, 0:w],
                                     w1[:, m * 128:(m + 1) * 128],
                                     gs[:, 0:w], start=True, stop=True)
                if zero_bias:
                    nc.scalar.activation(h1[:, mlo:mlo + 2, 0:w],
                                         z1[:, :, 0:w], Tanh, scale=inv_c)
                else:
                    for g in range(2):
                        m = mlo + g
                        nc.scalar.activation(h1[:, m, 0:w], z1[:, g, 0:w],
                                             Tanh, bias=b1[:, m:m + 1],
                                             scale=inv_c)
                nc.vector.tensor_sub(s[:, mlo:mlo + 2, 0:w],
                                     z1[:, :, 0:w], h1[:, mlo:mlo + 2, 0:w])

            def emit_l2_pair(bi, pair):
                ex, _, w = blocks[bi]
                _, w12, w2q, _, _, b2 = sb[ex]
                gs = gs_of[bi]
                s = s_of[bi]
                if pair == 0:
                    h2 = h2pool.tile([128, MCH, BLK], F16, tag="h2")
                    h2_of[bi] = h2
                h2 = h2_of[bi]
                # The last block's L2 borrows the z1 banks (free once its
                # own sub ran -- no L1 work remains), so the drain does
                # not serialize on ACT2(last-1) freeing a z2 buffer. The
                # manifest transform emits synthetic sync edges for the
                # extra slot reuse this creates.
                if bi == len(blocks) - 1:
                    z2 = z1pool.tile([128, 2, BLK], F32, tag="z1")
                else:
                    z2 = z2pool.tile([128, 2, BLK], F32, tag="z2")
                # Interleave the fp16 W12 matmuls with the fp8 DoubleRow
                # ones so DR weight loads can prefetch under other MMs.
                for g in range(2):
                    m = 2 * pair + g
                    nc.tensor.matmul(z2[:, g, 0:w],
                                     w12[:, m * 128:(m + 1) * 128],
                                     gs[:, 0:w], start=True, stop=False)
                for k2 in range(2):
                    for g in range(2):
                        m = 2 * pair + g
                        nc.tensor.matmul(
                            z2[:, g, 0:w],
                            w2q[k2][:, :, m * 128:(m + 1) * 128],
                            s[:, 2 * k2:2 * k2 + 2, 0:w],
                            start=False, stop=(k2 == 1), perf_mode=DR)
                mlo = 2 * pair
                if zero_bias:
                    nc.scalar.activation(h2[:, mlo:mlo + 2, 0:w],
                                         z2[:, :, 0:w], Tanh, scale=inv_w2sc)
                else:
                    for g in range(2):
                        m = mlo + g
                        nc.scalar.activation(h2[:, m, 0:w], z2[:, g, 0:w],
                                             Tanh, bias=b2[:, m:m + 1],
                                             scale=inv_w2sc)
                if pair == 1:
                    gs_of.pop(bi)
                    s_of.pop(bi)
                    z2_of[bi] = z2

            def emit_l3(bi):
                # M=1 matmuls packed 4-up in distinct 32-column PE groups
                # (tile_position); partial rows land on psum partitions
                # 0/32/64/96 and are summed on the host during unshard.
                # Output lands in the second z2 tile's upper half (already
                # consumed by ACT2b), so no dedicated PSUM bank is needed.
                ex, boff, w = blocks[bi]
                _, _, _, w3, _, _ = sb[ex]
                h2 = h2_of.pop(bi)
                z2b = z2_of.pop(bi)
                for k in range(MCH):
                    nc.tensor.matmul(
                        z2b[32 * k:32 * (k + 1), 1, 0:w],
                        w3[:, 32 * k:32 * (k + 1)],
                        h2[:, k, 0:w],
                        start=True, stop=True,
                        tile_position=(0, 32 * k))
                e_sb = epool.tile([97, BLK], F32, tag="e")
                nc.vector.tensor_copy(e_sb[:, 0:w], z2b[0:97, 1, 0:w])
                nc.sync.dma_start(e_out[:, boff:boff + w], e_sb[0:97:32, 0:w])

            # 3-stage software pipeline: L1(i) || L2(i-1) || L3(i-2).
            # L3 is emitted between L1 and L2 so that, on the PE stream,
            # nothing separates L2b(i-1) (ACT's last input of a step) from
            # L1p0(i+1) (ACT's next first input) — this closes a ~370ns
            # per-block ACT idle gap; L3's delay to L2a(i-1) is hidden
            # under the two ACT1 calls.
            nblocks = len(blocks)
            for i in range(nblocks + 2):
                for pair in range(2):
                    if i < nblocks:
                        emit_l1_pair(i, pair)
                if 0 <= i - 2 < nblocks:
                    emit_l3(i - 2)
                for pair in range(2):
                    if 0 <= i - 1 < nblocks:
                        emit_l2_pair(i - 1, pair)

    return len(blocks)




def _target_orders(nblocks):
    """Per-engine emission-order roles and the target hand schedule.
    PE: [L3(i-2) | L2(i-1) | L1(i+1)] per step closes the schedule at the
    ACT busy time; DVE: [ecopy(i-2), sub(i)p0, sub(i)p1] frees the z2
    tail bank before the next block's W12 needs it."""
    pe_em, dve_em, act_em = [], [], []
    for k in range(19):
        pe_em.append(("warm", k))
    for i in range(nblocks + 2):
        for pair in range(2):
            if i < nblocks:
                for g in range(2):
                    pe_em.append(("L1", i, pair, g))
                act_em.append(("A1", i, pair))
                dve_em.append(("sub", i, pair))
        if 0 <= i - 2 < nblocks:
            for k in range(4):
                pe_em.append(("L3", i - 2, k))
            dve_em.append(("ecopy", i - 2))
        for pair in range(2):
            if 0 <= i - 1 < nblocks:
                for g in range(2):
                    pe_em.append(("W12", i - 1, pair, g))
                for k2 in range(2):
                    for g in range(2):
                        pe_em.append(("DR", i - 1, pair, k2, g))
                act_em.append(("A2", i - 1, pair))
    pe_t, dve_t = [], []
    for k in range(5):
        pe_t.append(("warm", k))
    for pair in range(2):
        for g in range(2):
            pe_t.append(("L1", 0, pair, g))
    for k in range(5, 19):
        pe_t.append(("warm", k))
    if nblocks > 1:
        for pair in range(2):
            for g in range(2):
                pe_t.append(("L1", 1, pair, g))
    for i in range(nblocks + 3):
        if 0 <= i - 2 < nblocks:
            for k in range(4):
                pe_t.append(("L3", i - 2, k))
        if 0 <= i - 1 < nblocks:
            for pair in range(2):
                for g in range(2):
                    pe_t.append(("W12", i - 1, pair, g))
                for k2 in range(2):
                    for g in range(2):
                        pe_t.append(("DR", i - 1, pair, k2, g))
        if 2 <= i + 1 < nblocks:
            for pair in range(2):
                for g in range(2):
                    pe_t.append(("L1", i + 1, pair, g))
    for i in range(nblocks + 2):
        if 0 <= i - 2 < nblocks:
            dve_t.append(("ecopy", i - 2))
        if i < nblocks:
            for pair in range(2):
                dve_t.append(("sub", i, pair))
    assert sorted(map(str, pe_t)) == sorted(map(str, pe_em))
    assert sorted(map(str, dve_t)) == sorted(map(str, dve_em))
    return pe_em, dve_em, act_em, pe_t, dve_t, list(act_em)


def _transform_manifest(manifest_path, deps, out_path, nblocks, with_line):
    """Rewrite the captured flat schedule into the hand schedule, then
    legalize with a priority-guided topological sort over the dependency
    graph plus derived slot release->write edges (the replay validator
    requires a release of the previous tile in an address slot to
    happens-before every write of the next)."""
    import json as _json
    import heapq as _heapq
    d = _json.load(open(manifest_path))
    key = list(d["order"].keys())[0]
    order = d["order"][key]
    pe_em, dve_em, act_em, pe_t, dve_t, act_t = _target_orders(nblocks)

    desired = [dict(e) for e in order]
    for eng, em, tgt in (("PE", pe_em, pe_t), ("DVE", dve_em, dve_t),
                         ("Activation", act_em, act_t)):
        entries = [e for e in order if e["engine"] == eng]
        entries.sort(key=lambda e: int(e["name"].split("_")[0][2:]))
        assert len(entries) == len(em), (eng, len(entries), len(em))
        role2name = {str(r): e["name"] for r, e in zip(em, entries)}
        new_names = [role2name[str(r)] for r in tgt]
        it = iter(new_names)
        for e in desired:
            if e["engine"] == eng:
                e["name"] = next(it)

    prio = {e["name"]: float(i) for i, e in enumerate(desired)}
    entry_of = {e["name"]: e for e in desired}
    names = set(prio)

    def inum(n):
        return int(n.split("_")[0][2:])

    def all_preds(n):
        dd = deps.get(n, {})
        return [p for p in dd.get("pre_data", []) + dd.get("pre_no_sync", [])
                if p in names]

    releases = [e["name"] for e in desired
                if e["engine"] == "SP" and e.get("src", "").startswith(with_line)
                and "_alloc_" not in e["name"]]
    rel_of_tile = {}
    for r in releases:
        pd = deps.get(r, {}).get("pre_data", [])
        if pd:
            rel_of_tile[inum(min(pd, key=inum)) - 1] = r
    # Canonicalize PSUM z-tile addresses to strict slot alternation in
    # allocation order -- the legacy allocator may pack a block's two
    # pair tiles into one slot when ITS schedule's lifetimes allow,
    # which is unsatisfiable under the hand schedule's interleave. The
    # replayer honors manifest addresses, so rewriting them here both
    # fixes that and keeps the slot-edge derivation uniform.
    regions = {}
    for n, (addr, space) in d["addresses"].items():
        if space == "PSUM":
            regions.setdefault(addr // 8192, []).append(n)
    for reg, tn in regions.items():
        tn.sort(key=lambda n: int(n.rsplit("_", 1)[1]))
        taddrs = sorted({d["addresses"][n][0] for n in tn})
        if len(taddrs) == 2:
            for i, n in enumerate(tn):
                d["addresses"][n][0] = taddrs[i % 2]
    slots = {}
    for tname, (addr, space) in d["addresses"].items():
        slots.setdefault((addr, space), []).append(int(tname.rsplit("_", 1)[1]))
    preds = {n: set(all_preds(n)) for n in names}
    for nums in slots.values():
        nums.sort()
        for old, new in zip(nums, nums[1:]):
            r_old, r_new = rel_of_tile.get(old), rel_of_tile.get(new)
            if r_old is None or r_new is None:
                continue
            for w in deps.get(r_new, {}).get("pre_data", []):
                if w in names and r_old != w:
                    preds[w].add(r_old)
    for _ in range(3):
        for r in releases:
            ps = all_preds(r)
            if ps:
                prio[r] = max(prio[p] for p in ps) + 0.001
    succs = {}
    for n, ps in preds.items():
        for p in ps:
            succs.setdefault(p, set()).add(n)
    indeg = {n: len(preds[n]) for n in names}
    heap = [(prio[n], n) for n in names if indeg[n] == 0]
    _heapq.heapify(heap)
    out = []
    while heap:
        _, n = _heapq.heappop(heap)
        out.append(entry_of[n])
        for s in succs.get(n, ()):
            indeg[s] -= 1
            if indeg[s] == 0:
                _heapq.heappush(heap, (prio[s], s))
    assert len(out) == len(desired), "cycle in legalized schedule"

    # Emit synthetic (cross-engine sync) edges for every slot-WAR
    # requirement the replayer cannot prove transitively: the validator
    # needs release(old tile) to happens-before each write of the next
    # tile in the same address slot, where happens-before = data deps +
    # same-engine queue order + synthetic deps. Compute ancestor sets
    # over that graph in flat order and patch the gaps.
    pos = {e["name"]: i for i, e in enumerate(out)}
    known_preds = {n: set() for n in names}
    for n in names:
        dd = deps.get(n, {})
        for p in dd.get("pre_data", []) + dd.get("pre_no_sync", []):
            if p in names:
                known_preds[n].add(p)
    last_on_engine = {}
    for e in out:
        eng = e["engine"]
        if eng in last_on_engine:
            known_preds[e["name"]].add(last_on_engine[eng])
        last_on_engine[eng] = e["name"]

    bit = {e["name"]: 1 << i for i, e in enumerate(out)}
    required = []  # (accessor_of_old_tile, member_of_new, pos_of_member)
    for nums in slots.values():
        for old_t, new_t in zip(nums, nums[1:]):
            r_old = rel_of_tile.get(old_t)
            r_new = rel_of_tile.get(new_t)
            if r_old is None or r_new is None:
                continue
            accs = [a for a in deps.get(r_old, {}).get("pre_data", [])
                    if a in names]
            for w in deps.get(r_new, {}).get("pre_data", []):
                if w in names:
                    for a in accs:
                        if a != w:
                            required.append((a, w, pos[w]))
    required.sort(key=lambda t: t[2])
    anc = {}
    synth = []
    req_i = 0
    for e in out:
        n = e["name"]
        a = 0
        for p in known_preds[n]:
            a |= anc[p] | bit[p]
        while req_i < len(required) and required[req_i][2] == pos[n]:
            acc = required[req_i][0]
            req_i += 1
            if not (a & bit[acc]) and pos[acc] < pos[n]:
                synth.append([acc, n])
                a |= anc[acc] | bit[acc]
        anc[n] = a
    d["synthetic_dependencies"] = list(d.get("synthetic_dependencies", [])) + synth

    d["order"][key] = out
    _json.dump(d, open(out_path, "w"))


def _build_program(n_a: int, n_b: int, zero_bias: bool):
    """Compile twice: once under the legacy scheduler to capture its
    manifest (cost-model clocks biased so the sim places PE/DVE work
    realistically), then transform the flat order into the hand schedule
    and replay it via the manifest scheduler. Falls back to the plain
    legacy compile if any step fails."""
    import os
    import json
    import shutil
    import tempfile
    key = (n_a, n_b, zero_bias)
    if key in _PROGRAM_CACHE:
        return _PROGRAM_CACHE[key]

    import concourse.hw_specs as _hw_specs
    import concourse.manifest_helpers as _mh
    import concourse.tile as _tile_mod

    def _biased_emit(nc):
        dve = mybir.EngineType.DVE
        orig_pe = _hw_specs.TRN2Spec.PE_CYCLE
        orig_dve = _hw_specs.TRN2Spec.CYCLE_T[dve]
        _hw_specs.TRN2Spec.PE_CYCLE = 1e9 / 1.6e9
        _hw_specs.TRN2Spec.CYCLE_T[dve] = 1e9 / 0.83e9
        try:
            return _emit(nc, n_a, n_b, zero_bias)
        finally:
            _hw_specs.TRN2Spec.PE_CYCLE = orig_pe
            _hw_specs.TRN2Spec.CYCLE_T[dve] = orig_dve

    capdir = tempfile.mkdtemp(prefix="bp_mcap_")
    loaddir = tempfile.mkdtemp(prefix="bp_mload_")
    saved_env = {k: os.environ.get(k) for k in
                 ("TILE_SCHEDULER", "TILE_LOAD_MANIFEST_PATH",
                  "TILE_CAPTURE_MANIFEST_PATH")}

    def _restore_env():
        for k, v in saved_env.items():
            if v is None:
                os.environ.pop(k, None)
            else:
                os.environ[k] = v

    captured = {}

    def _capture_patched(tc, capture_path, ordered, sched_state, pre_deps,
                         init_addrs):
        manifest = _mh.capture_manifest(ordered, sched_state, tc.tiles,
                                        tc._perfetto_entries, tc.nc,
                                        pre_deps, init_addrs)
        path = str(capture_path)
        with open(path, "w") as f:
            f.write(_mh.dump_manifest(manifest))
        captured["path"] = path
        captured["deps"] = _mh.build_standalone_instruction_deps_map(
            ordered, pre_deps)

    def _load_patched(path):
        with open(str(path), "r") as f:
            return f.read()

    nc = None
    try:
        orig_cap = _mh.capture_and_write_manifest
        orig_tile_cap = _tile_mod.capture_and_write_manifest
        orig_load = _mh.load_manifest
        _mh.capture_and_write_manifest = _capture_patched
        _tile_mod.capture_and_write_manifest = _capture_patched
        _mh.load_manifest = _load_patched
        try:
            os.environ.pop("TILE_SCHEDULER", None)
            os.environ.pop("TILE_LOAD_MANIFEST_PATH", None)
            os.environ["TILE_CAPTURE_MANIFEST_PATH"] = capdir
            nc_cap = bacc.Bacc("TRN2", target_bir_lowering=False,
                               debug=False, num_devices=N_CORES)
            nblocks = _emit(nc_cap, n_a, n_b, zero_bias)
            del nc_cap
            assert "path" in captured, "manifest capture did not run"

            lineno = None
            with open(os.path.abspath(__file__)) as f:
                for ln, text in enumerate(f, 1):
                    if "with tile.TileContext(nc) as tc:" in text:
                        lineno = ln
                        break
            base = os.path.basename(__file__)
            with_line = f"{base}:{lineno} "

            out_path = os.path.join(loaddir,
                                    os.path.basename(captured["path"]))
            _transform_manifest(captured["path"], captured["deps"],
                                out_path, nblocks, with_line)

            os.environ.pop("TILE_CAPTURE_MANIFEST_PATH", None)
            os.environ["TILE_SCHEDULER"] = "manifest"
            os.environ["TILE_LOAD_MANIFEST_PATH"] = loaddir
            nc = bacc.Bacc("TRN2", target_bir_lowering=False, debug=False,
                           num_devices=N_CORES)
            _emit(nc, n_a, n_b, zero_bias)
            nc.compile()
        finally:
            _mh.capture_and_write_manifest = orig_cap
            _tile_mod.capture_and_write_manifest = orig_tile_cap
            _mh.load_manifest = orig_load
            _restore_env()
    except Exception:
        if os.environ.get("BP_STRICT"):
            raise
        # Any failure in the manifest flow: plain legacy compile.
        _restore_env()
        nc = bacc.Bacc("TRN2", target_bir_lowering=False, debug=False,
                       num_devices=N_CORES)
        _biased_emit(nc)
        nc.compile()
    finally:
        shutil.rmtree(capdir, ignore_errors=True)
        shutil.rmtree(loaddir, ignore_errors=True)

    _PROGRAM_CACHE[key] = nc
    return nc


def kernel(**inputs) -> np.ndarray:
    Gs = np.ascontiguousarray(np.asarray(inputs["Gs"], dtype=np.float32))
    types = np.asarray(inputs["types"])
    mol_id = np.asarray(inputs["mol_id"])
    n_atoms = Gs.shape[0]

    idx = [np.flatnonzero(types == 0), np.flatnonzero(types != 0)]
    # Per-core atom counts (equal across cores for SPMD; pad with zeros).
    GRAN = 128
    n_a, n_b = (int(-(-len(ix) // (N_CORES * GRAN))) * GRAN for ix in idx)
    npc = n_a + n_b

    GsT = Gs.astype(np.float16).T  # [128, N] fp16 feature-major view

    wk = {}
    bias_mag = 0.0
    for t, pre in (("a", "A"), ("b", "B")):
        W1 = np.asarray(inputs[f"W1_{pre}"], np.float64)
        W2 = np.asarray(inputs[f"W2_{pre}"], np.float64)
        b1 = np.asarray(inputs[f"b1_{pre}"], np.float64).reshape(-1)
        b2 = np.asarray(inputs[f"b2_{pre}"], np.float64).reshape(-1)
        bias_mag = max(bias_mag, np.abs(b1).max(initial=0.0),
                       np.abs(b2).max(initial=0.0))
        wk[f"w1{t}"] = np.ascontiguousarray((C_LIN * W1).astype(np.float16))
        wk[f"w12{t}"] = np.ascontiguousarray(
            (W2SC * ((C_LIN * W1) @ W2)).astype(np.float16))
        for k2 in range(2):
            # w2q[p, i, m] = Q(-W2SC * W2[k2*256 + i*128 + p, m])
            blk = -W2SC * W2[k2 * 256:(k2 + 1) * 256, :]      # [256, 512]
            wk[f"w2q{k2}{t}"] = np.ascontiguousarray(
                _q8(blk.reshape(2, 128, HIDDEN).transpose(1, 0, 2)))
        w3chunks = np.asarray(
            inputs[f"W3_{pre}"], np.float32)[:, 0].reshape(MCH, 128).T
        w3p = np.zeros((128, 32 * MCH), np.float16)
        w3p[:, 0::32] = w3chunks.astype(np.float16)
        wk[f"w3{t}"] = w3p
        wk[f"b1{t}"] = np.ascontiguousarray(
            b1.astype(np.float32).reshape(MCH, 128).T)
        wk[f"b2{t}"] = np.ascontiguousarray(
            b2.astype(np.float32).reshape(MCH, 128).T)
        wk[f"b3{t}"] = np.float32(
            np.asarray(inputs[f"b3_{pre}"], np.float32).reshape(())
            + np.asarray(inputs[f"off_{pre}"], np.float32).reshape(()))

    zero_bias = bias_mag == 0.0
    send = {k: v for k, v in wk.items()
            if not k.startswith("b3") and not (
                zero_bias and (k.startswith("b1") or k.startswith("b2")))}

    chunks = []  # per core: (a_indices, b_indices)
    in_maps = []
    for i in range(N_CORES):
        ca = idx[0][i * n_a:(i + 1) * n_a]
        cb = idx[1][i * n_b:(i + 1) * n_b]
        chunks.append((ca, cb))
        buf = np.zeros((NUM_GS, npc), np.float16)
        buf[:, :len(ca)] = GsT[:, ca]
        buf[:, n_a:n_a + len(cb)] = GsT[:, cb]
        in_maps.append({"gst": buf, **send})

    nc = _build_program(n_a, n_b, zero_bias)
    results = None
    for attempt in range(3):
        try:
            results = run_bass_kernel_spmd(
                nc, in_maps, list(range(N_CORES))).results
            break
        except Exception:
            # Transient NRT/device hiccups usually clear on retry.
            if attempt == 2:
                raise
            time.sleep(2.0)

    e = np.empty(n_atoms, np.float32)
    for i in range(N_CORES):
        r = np.asarray(results[i]["e_out"]).sum(axis=0, dtype=np.float32)
        ca, cb = chunks[i]
        e[ca] = r[:len(ca)] + wk["b3a"]
        e[cb] = r[n_a:n_a + len(cb)] + wk["b3b"]

    sums = np.bincount(mol_id, weights=e.astype(np.float64),
                       minlength=N_MOL)[:N_MOL]
    counts = np.bincount(mol_id, minlength=N_MOL)[:N_MOL]
    out = sums / np.maximum(counts, 1)
    return out.astype(np.float32)[:, None]

